# revision 9
# baseline (speedup 1.0000x reference)
"""Trainium2 Bass kernel for causal multi-head attention.

Shapes (hardcoded): B=4, T=2048, D=1024, H=16, Dh=64, fp32 I/O.

Strategy (8 NeuronCores, tensor-parallel over heads):
  - Each core c owns heads (2c, 2c+1): computes Q^T/K^T/V projections for its
    128 head-dims over the whole [B*T, D] input (contracting D on the PE),
    then causal flash-style attention in "scores-transposed" orientation
    (S^T[k, q] blocks) so softmax needs no on-chip transposes:
      * exp on ScalarE, one merged instruction per key-block covering both
        heads ([128, 2, width] over a 2-bank PSUM group)
      * causal handling at 128-column granularity: S matmuls, exps and AV
        matmuls of the 4 diagonal blocks of each q-chunk are narrowed to the
        live query range; only the 128x128 boundary triangle gets a mask
      * denominator via a leading ones-column in the V stationary operand
        (row 0 of the AV psum = sum of exp weights)
      * division folded into the PSUM->SBUF cast against a PE-broadcast
        reciprocal
  - K^T is stored zero-padded per head ([128, 2, BT]) so every matmul in the
    kernel runs in the PE's 128x128 tile mode (no tiling-mode switches).
  - Projection / out-projection matmul groups are emitted as *filler* between
    attention blocks so the PE never idles (sustains the 2.4 GHz p-state).
  - An on-device AllToAll re-shards ctx^T from head-sharded to row-sharded,
    then each core computes out rows = ctx @ Wo + bo.

All matmul operands are fp16; accumulation is fp32 in PSUM.
"""

import sys

sys.path.insert(0, "/opt/trn_rl_repo")

import numpy as np

import concourse.bass as bass
import concourse.mybir as mybir
import concourse.tile as tile
from concourse import bacc
from concourse import bass_utils

N_CORES = 8
B, T, D, H, DH = 4, 2048, 1024, 16, 64
BT = B * T  # 8192
KS = D // 128  # 8 contraction subtiles
TC = 512  # t-chunk for projections
NTC = BT // TC  # 16
QC = 512  # query chunk in attention
NQC = T // QC  # 4 per batch
KB = 128  # key block
NKB = T // KB  # 16 per batch
ROWS = BT // N_CORES  # 1024 out rows per core
RB4 = ROWS // B  # 256 out rows per core per batch

F16 = mybir.dt.float16
F32 = mybir.dt.float32

_CACHE = {}


def _build():
    nc = bacc.Bacc("TRN2", target_bir_lowering=False, num_devices=N_CORES)

    x_d = nc.dram_tensor("x", [D, BT], F16, kind="ExternalInput")  # pre-transposed
    wq_d = nc.dram_tensor("wq", [D, 128], F16, kind="ExternalInput")
    wk_d = nc.dram_tensor("wk", [D, 128], F16, kind="ExternalInput")
    wv_d = nc.dram_tensor("wv", [D, 128], F16, kind="ExternalInput")
    wo_d = nc.dram_tensor("wo", [D, D], F16, kind="ExternalInput")
    bo_d = nc.dram_tensor("bo", [D], F32, kind="ExternalInput")
    e2_d = nc.dram_tensor("e2", [128, 128], F16, kind="ExternalInput")
    cmask_d = nc.dram_tensor("cmask", [128, 2, 128], F16, kind="ExternalInput")
    out_d = nc.dram_tensor("out", [B, RB4, D], F32, kind="ExternalOutput")

    with tile.TileContext(nc) as tc:
        with (
            tc.tile_pool(name="persist", bufs=1) as persist,
            tc.tile_pool(name="xt", bufs=3) as xtp,
            tc.tile_pool(name="ep", bufs=5) as ep,
            tc.tile_pool(name="tail", bufs=2) as tailp,
            tc.tile_pool(name="ctx", bufs=3) as ctxp,
            tc.tile_pool(name="outp", bufs=3) as outp,
            tc.tile_pool(name="ps_s", bufs=2, space="PSUM") as ps_s,
            tc.tile_pool(name="ps_av", bufs=3, space="PSUM") as ps_av,
            tc.tile_pool(name="ps_misc", bufs=1, space="PSUM") as ps_misc,
            tc.tile_pool(name="dram", bufs=1, space="DRAM") as dram,
        ):
            # ---- persistent state ----
            wq_sb = persist.tile([128, KS, 128], F16)
            wk_sb = persist.tile([128, KS, 128], F16)
            wv_sb = persist.tile([128, KS, 128], F16)
            wo_sb = persist.tile([128, KS, D], F16)
            nc.sync.dma_start(wq_sb[:], wq_d.rearrange("(o p) h -> p o h", p=128))
            nc.sync.dma_start(wk_sb[:], wk_d.rearrange("(o p) h -> p o h", p=128))
            nc.sync.dma_start(wv_sb[:], wv_d.rearrange("(o p) h -> p o h", p=128))
            nc.sync.dma_start(wo_sb[:], wo_d.rearrange("(r p) n -> p r n", p=128))

            qt_sb = persist.tile([128, BT], F16)  # [2 heads x 64, global t]
            # K^T zero-padded per head: [:, 0, t] rows 0-63 = head0 (rest 0),
            # [:, 1, t] rows 64-127 = head1 (rest 0)
            ktp_sb = persist.tile([128, 2, BT], F16)
            # V layout: [128 keys-in-block, B*NKB blocks, 2*(1+64)]
            #   per head h: cols 0:64 = V_h, col 64 = ones (denominator)
            v_sb = persist.tile([128, B * NKB, 2, DH + 1], F16)
            nc.vector.memset(v_sb[:, :, :, DH : DH + 1], 1.0)

            # per-partition selector scales for the padded K^T casts
            s01 = persist.tile([128, 1], F32)
            s10 = persist.tile([128, 1], F32)
            nc.vector.memset(s01[0:64], 1.0)
            nc.vector.memset(s01[64:128], 0.0)
            nc.vector.memset(s10[0:64], 0.0)
            nc.vector.memset(s10[64:128], 1.0)

            # bias broadcast [128, D] fp32 via PE ones-trick
            ones_col = persist.tile([1, 128], F32)
            nc.vector.memset(ones_col[:], 1.0)
            bo_sb = persist.tile([1, D], F32)
            nc.sync.dma_start(bo_sb[:], bo_d[None, :])
            bias_sb = persist.tile([128, D], F32)
            for nch in range(2):
                bps = ps_misc.tile([128, 512], F32, tag="misc")
                nc.tensor.matmul(
                    bps[:], ones_col[:], bo_sb[:, nch * 512 : (nch + 1) * 512]
                )
                nc.vector.tensor_copy(bias_sb[:, nch * 512 : (nch + 1) * 512], bps[:])

            # padded E2 selector (rows 0-63 <- r2[0], 64-127 <- r2[1]; rows
            # 2-127 of the moving operand are zero)
            e2_sb = persist.tile([128, 128], F16)
            nc.sync.dma_start(e2_sb[:], e2_d[:])
            r2hp = persist.tile([128, QC], F16)
            nc.vector.memset(r2hp[:], 0.0)

            # boundary triangle mask (both heads): cmask[p, h, j] = (j >= p)
            cmask_sb = persist.tile([128, 2, 128], F16)
            nc.sync.dma_start(cmask_sb[:], cmask_d[:])

            # ---- projection emission (as filler items) ----
            def emit_xt_dma(tcn):
                t0 = tcn * TC
                xt = xtp.tile([128, KS, TC], F16, tag="xt", name="xt")
                nc.sync.dma_start(
                    xt[:],
                    x_d[:, t0 : t0 + TC].rearrange("(o p) t -> p o t", p=128),
                )
                return xt

            def emit_q_group(xt, tcn):
                t0 = tcn * TC
                pp = ps_misc.tile([128, TC], F32, tag="misc", name="qp")
                for ks in range(KS):
                    nc.tensor.matmul(
                        pp[:], wq_sb[:, ks, :], xt[:, ks, :],
                        start=(ks == 0), stop=(ks == KS - 1),
                    )
                nc.scalar.copy(qt_sb[:, t0 : t0 + TC], pp[:])

            def emit_k_group(xt, tcn):
                t0 = tcn * TC
                pp = ps_misc.tile([128, TC], F32, tag="misc", name="kp")
                for ks in range(KS):
                    nc.tensor.matmul(
                        pp[:], wk_sb[:, ks, :], xt[:, ks, :],
                        start=(ks == 0), stop=(ks == KS - 1),
                    )
                nc.vector.tensor_scalar_mul(ktp_sb[:, 0, t0 : t0 + TC], pp[:], s01[:])
                nc.vector.tensor_scalar_mul(ktp_sb[:, 1, t0 : t0 + TC], pp[:], s10[:])

            def emit_v_sub(xt, tcn, sub):
                vp = ps_misc.tile([128, 2, DH], F32, tag="misc", name="vp")
                for ks in range(KS):
                    nc.tensor.matmul(
                        vp[:],
                        xt[:, ks, sub * 128 : (sub + 1) * 128],
                        wv_sb[:, ks, :],
                        start=(ks == 0), stop=(ks == KS - 1),
                    )
                kbg = tcn * (TC // 128) + sub
                dst = v_sb[:, kbg, :, 0:DH]  # cols {0..63} u {65..128}
                nc.vector.tensor_copy(dst, vp[:])

            def proj_chunk_items(tcn):
                state = {}

                def first():
                    state["xt"] = emit_xt_dma(tcn)
                    emit_q_group(state["xt"], tcn)

                items = [first]
                items.append(lambda: emit_k_group(state["xt"], tcn))
                for sub in range(TC // 128):
                    items.append(
                        lambda s=sub: emit_v_sub(state["xt"], tcn, s)
                    )
                return items

            # ---- out-projection (as filler items) ----
            cc_ins = [dram.tile([N_CORES, 128, RB4], F16, name=f"cc_in{b}", tag=f"cc_in{b}") for b in range(B - 1)]
            cc_outs = [dram.tile([N_CORES, 128, RB4], F16, name=f"cc_out{b}", tag=f"cc_out{b}") for b in range(B - 1)]
            cc_ins_h = [dram.tile([N_CORES, 128, RB4 // 2], F16, name=f"cc_inh{i}", tag=f"cc_inh{i}") for i in range(2)]
            cc_outs_h = [dram.tile([N_CORES, 128, RB4 // 2], F16, name=f"cc_outh{i}", tag=f"cc_outh{i}") for i in range(2)]
            ao_sbs = []

            def emit_oproj_group(item):
                ob, oao, mb, nch = item
                t_in_ao = (mb * 128) % oao.shape[2]
                op = ps_misc.tile([128, 512], F32, tag="misc", name="op")
                for r in range(KS):
                    nc.tensor.matmul(
                        op[:],
                        oao[:, r, t_in_ao : t_in_ao + 128],
                        wo_sb[:, r, nch * 512 : (nch + 1) * 512],
                        start=(r == 0), stop=(r == KS - 1),
                    )
                osb = outp.tile([128, 512], F32, tag="osb", name="osb")
                nc.vector.tensor_tensor(
                    osb[:], op[:], bias_sb[:, nch * 512 : (nch + 1) * 512],
                    mybir.AluOpType.add,
                )
                nc.sync.dma_start(
                    out_d[ob, mb * 128 : (mb + 1) * 128,
                          nch * 512 : (nch + 1) * 512],
                    osb[:],
                )

            # ---- filler queue: keeps the PE fed between attention blocks ----
            filler = []

            def pop_filler(n=1):
                for _ in range(n):
                    if filler:
                        filler.pop(0)()

            # batch 0 projections (+ chunk 4) emitted up front
            for tcn in range(5):
                for it in proj_chunk_items(tcn):
                    it()
            for tcn in range(5, NTC):
                filler.extend(proj_chunk_items(tcn))

            def emit_half_a2a(half):
                nc.gpsimd.collective_compute(
                    "AllToAll",
                    mybir.AluOpType.bypass,
                    replica_groups=[list(range(N_CORES))],
                    ins=[cc_ins_h[half][:]],
                    outs=[cc_outs_h[half][:]],
                )
                RBH = RB4 // 2
                ao_sb = persist.tile([128, KS, RBH], F16, name=f"aoh{half}", tag=f"aoh{half}")
                ao_sbs.append(ao_sb)
                nc.sync.dma_start(ao_sb[:], cc_outs_h[half].rearrange("r p t -> p r t"))
                for nch in range(2):
                    filler.append(
                        lambda a=ao_sb, h=half, n=nch: emit_oproj_group((B - 1, a, h, n))
                    )

            # ---- attention ----
            pending_tail = []

            def emit_qc_tail(b, qc, av0, av1):
                # reciprocals of denominators (row 64 of each av bank);
                # reciprocal_approx_fast requires partition-0 operands
                d2a = tailp.tile([1, QC], F32, tag="d2a")
                d2b = tailp.tile([1, QC], F32, tag="d2b")
                nc.vector.tensor_copy(d2a[:], av0[DH : DH + 1, :])
                nc.vector.tensor_copy(d2b[:], av1[DH : DH + 1, :])
                r2a = tailp.tile([1, QC], F32, tag="r2a")
                r2b = tailp.tile([1, QC], F32, tag="r2b")
                nc.vector.reciprocal_approx_fast(r2a[:], d2a[:])
                nc.vector.reciprocal_approx_fast(r2b[:], d2b[:])
                nc.vector.tensor_copy(r2hp[0:1, :], r2a[:])
                nc.vector.tensor_copy(r2hp[64:65, :], r2b[:])
                rb = ps_misc.tile([128, QC], F32, tag="misc", name="rb")
                nc.tensor.matmul(rb[:], e2_sb[:], r2hp[:])
                rb_sb = tailp.tile([128, QC], F16, tag="rbs")
                nc.vector.tensor_copy(rb_sb[:], rb[:])
                ctx2 = ctxp.tile([128, QC], F16, tag="ctx")
                nc.vector.tensor_tensor(
                    ctx2[0:64, :], av0[0:DH, :], rb_sb[0:64, :],
                    mybir.AluOpType.mult,
                )
                nc.vector.tensor_tensor(
                    ctx2[64:128, :], av1[0:DH, :], rb_sb[64:128, :],
                    mybir.AluOpType.mult,
                )
                if b < B - 1:
                    s0 = qc * QC // RB4
                    nc.sync.dma_start(
                        cc_ins[b][s0 : s0 + QC // RB4].rearrange("s p f -> p s f"),
                        ctx2[:].rearrange("p (s f) -> p s f", s=QC // RB4),
                    )
                else:
                    half, RBH = qc // 2, RB4 // 2
                    s0 = (qc % 2) * QC // RBH
                    nc.sync.dma_start(
                        cc_ins_h[half][s0 : s0 + QC // RBH].rearrange("s p f -> p s f"),
                        ctx2[:].rearrange("p (s f) -> p s f", s=QC // RBH),
                    )

            for b in range(B):
                for qc in range(NQC):
                    q0 = b * T + qc * QC
                    nkb = 4 * qc + 4

                    def emit_se(kb):
                        # scores + merged exp (+ triangle mask) for block kb
                        k0 = b * T + kb * KB
                        diag_i = kb - 4 * qc  # >= 0 for diagonal blocks
                        lo = max(0, diag_i) * 128  # live query range start
                        sp = ps_s.tile([128, 2, QC], F32, tag="s", name="sp")
                        for h in (0, 1):
                            nc.tensor.matmul(
                                sp[:, h, lo:QC],
                                ktp_sb[:, h, k0 : k0 + KB],
                                qt_sb[:, q0 + lo : q0 + QC],
                            )
                        e = ep.tile([128, 2, QC], F16, tag="e", name="e")
                        nc.scalar.activation(
                            e[:, :, lo:QC], sp[:, :, lo:QC],
                            mybir.ActivationFunctionType.Exp,
                            scale=0.125,
                        )
                        if diag_i >= 0:
                            nc.gpsimd.tensor_tensor(
                                e[:, :, lo : lo + 128],
                                e[:, :, lo : lo + 128],
                                cmask_sb[:],
                                mybir.AluOpType.mult,
                            )
                        return e, lo

                    # warm the S/exp pipeline, then flush the previous
                    # q-chunk's tail (reads its av psum) BEFORE allocating
                    # this q-chunk's av tiles from the rotating pool
                    e_q = [emit_se(0)]
                    if nkb > 1:
                        e_q.append(emit_se(1))
                    while pending_tail:
                        pending_tail.pop(0)()
                    if b == B - 1 and qc == 2:
                        emit_half_a2a(0)

                    av0_full = ps_av.tile([128, QC], F32, tag="av", name="av0")
                    av1_full = ps_av.tile([128, QC], F32, tag="av", name="av1")
                    av0 = av0_full[: DH + 1]
                    av1 = av1_full[: DH + 1]

                    for kb in range(nkb):
                        kbg = b * NKB + kb
                        first, last = kb == 0, kb == nkb - 1
                        e_cur, lo = e_q.pop(0)
                        if kb + 2 < nkb:
                            e_q.append(emit_se(kb + 2))
                        for h, av in ((0, av0), (1, av1)):
                            nc.tensor.matmul(
                                av[:, lo:QC], v_sb[:, kbg, h, :], e_cur[:, h, lo:QC],
                                start=first, stop=last,
                            )
                        if kb % 2 == 1:
                            pop_filler(1)
                    pending_tail.append(
                        lambda b_=b, qc_=qc, a0=av0_full, a1=av1_full: emit_qc_tail(
                            b_, qc_, a0, a1
                        )
                    )
                    pop_filler(1)

                # ---- per-batch all-to-all; out-proj groups become filler ----
                if b < B - 1:
                    while pending_tail:
                        pending_tail.pop(0)()
                    nc.gpsimd.collective_compute(
                        "AllToAll",
                        mybir.AluOpType.bypass,
                        replica_groups=[list(range(N_CORES))],
                        ins=[cc_ins[b][:]],
                        outs=[cc_outs[b][:]],
                    )
                    ao_sb = persist.tile([128, KS, RB4], F16, name=f"ao{b}", tag=f"ao{b}")
                    ao_sbs.append(ao_sb)
                    nc.sync.dma_start(ao_sb[:], cc_outs[b].rearrange("r p t -> p r t"))
                    for mb in range(RB4 // 128):
                        for nch in range(2):
                            filler.append(
                                lambda a=ao_sb, m=mb, n=nch, bb=b: emit_oproj_group(
                                    (bb, a, m, n)
                                )
                            )

            while pending_tail:
                pending_tail.pop(0)()
            emit_half_a2a(1)
            while filler:
                filler.pop(0)()

    nc.compile()
    return nc


def _get_nc():
    if "nc" not in _CACHE:
        _CACHE["nc"] = _build()
    return _CACHE["nc"]


def prepare_in_maps(x, Wq, Wk, Wv, Wo, bo):
    x16 = np.ascontiguousarray(np.asarray(x, dtype=np.float32).reshape(BT, D).T).astype(np.float16)
    wo16 = np.asarray(Wo, dtype=np.float32).astype(np.float16)
    bo32 = np.ascontiguousarray(np.asarray(bo, dtype=np.float32))
    e2 = np.zeros((128, 128), dtype=np.float16)
    e2[0, 0:64] = 1.0
    e2[64, 64:128] = 1.0
    p = np.arange(128)[:, None]
    j = np.arange(128)[None, :]
    cmask = np.broadcast_to((j >= p).astype(np.float16)[:, None, :], (128, 2, 128))
    cmask = np.ascontiguousarray(cmask)
    in_maps = []
    for c in range(N_CORES):
        cs = slice(128 * c, 128 * (c + 1))
        in_maps.append(
            {
                "x": x16,
                "wq": np.ascontiguousarray(np.asarray(Wq, np.float32)[:, cs]).astype(np.float16),
                "wk": np.ascontiguousarray(np.asarray(Wk, np.float32)[:, cs]).astype(np.float16),
                "wv": np.ascontiguousarray(np.asarray(Wv, np.float32)[:, cs]).astype(np.float16),
                "wo": wo16,
                "bo": bo32,
                "e2": e2,
                "cmask": cmask,
            }
        )
    return in_maps


def kernel(x, Wq, Wk, Wv, Wo, bo, _trace=False):
    nc = _get_nc()
    in_maps = prepare_in_maps(x, Wq, Wk, Wv, Wo, bo)
    res = bass_utils.run_bass_kernel_spmd(
        nc, in_maps, list(range(N_CORES)), trace=_trace
    )
    if _trace:
        _CACHE["last_results"] = res
    out = np.empty((B, T, D), dtype=np.float32)
    rb4 = ROWS // B
    rbh = rb4 // 2
    for c in range(N_CORES):
        oc = res.results[c]["out"]  # [B, 256, D]
        for b in range(B - 1):
            out[b, rb4 * c : rb4 * (c + 1), :] = oc[b]
        # last batch was exchanged as two half-batch A2As with 128-row shards
        out[B - 1, rbh * c : rbh * (c + 1), :] = oc[B - 1, 0:rbh]
        out[B - 1, T // 2 + rbh * c : T // 2 + rbh * (c + 1), :] = oc[B - 1, rbh:]
    return out


# revision 12
# speedup vs baseline: 1.1121x; 1.1121x over previous
"""Trainium2 Bass kernel for causal multi-head attention.

Shapes (hardcoded): B=4, T=2048, D=1024, H=16, Dh=64, fp32 I/O.

Strategy (8 NeuronCores, tensor-parallel over heads):
  - Each core c owns heads (2c, 2c+1): computes Q^T/K^T/V projections for its
    128 head-dims over the whole [B*T, D] input (contracting D on the PE),
    then causal flash-style attention in "scores-transposed" orientation
    (S^T[k, q] blocks) so softmax needs no on-chip transposes:
      * exp on ScalarE, one merged instruction per key-block covering both
        heads ([128, 2, width] over a 2-bank PSUM group)
      * causal handling at 128-column granularity: S matmuls, exps and AV
        matmuls of the 4 diagonal blocks of each q-chunk are narrowed to the
        live query range; only the 128x128 boundary triangle gets a mask
      * denominator via a leading ones-column in the V stationary operand
        (row 0 of the AV psum = sum of exp weights)
      * division folded into the PSUM->SBUF cast against a PE-broadcast
        reciprocal
  - K^T is stored zero-padded per head ([128, 2, BT]) so every matmul in the
    kernel runs in the PE's 128x128 tile mode (no tiling-mode switches).
  - Projection / out-projection matmul groups are emitted as *filler* between
    attention blocks so the PE never idles (sustains the 2.4 GHz p-state).
  - An on-device AllToAll re-shards ctx^T from head-sharded to row-sharded,
    then each core computes out rows = ctx @ Wo + bo.

All matmul operands are fp16; accumulation is fp32 in PSUM.
"""

import sys

sys.path.insert(0, "/opt/trn_rl_repo")

import numpy as np

import concourse.bass as bass
import concourse.mybir as mybir
import concourse.tile as tile
from concourse import bacc
from concourse import bass_utils

N_CORES = 8
B, T, D, H, DH = 4, 2048, 1024, 16, 64
BT = B * T  # 8192
KS = D // 128  # 8 contraction subtiles
TC = 512  # t-chunk for projections
NTC = BT // TC  # 16
QC = 512  # query chunk in attention
NQC = T // QC  # 4 per batch
KB = 128  # key block
NKB = T // KB  # 16 per batch
ROWS = BT // N_CORES  # 1024 out rows per core
RB4 = ROWS // B  # 256 out rows per core per batch

F16 = mybir.dt.float16
F32 = mybir.dt.float32

_CACHE = {}


def _build():
    nc = bacc.Bacc("TRN2", target_bir_lowering=False, num_devices=N_CORES)

    x_d = nc.dram_tensor("x", [D, BT], F16, kind="ExternalInput")  # pre-transposed
    wq_d = nc.dram_tensor("wq", [D, 128], F16, kind="ExternalInput")
    wk_d = nc.dram_tensor("wk", [D, 128], F16, kind="ExternalInput")
    wv_d = nc.dram_tensor("wv", [D, 128], F16, kind="ExternalInput")
    wo_d = nc.dram_tensor("wo", [D, D], F16, kind="ExternalInput")
    bo_d = nc.dram_tensor("bo", [D], F32, kind="ExternalInput")
    e2_d = nc.dram_tensor("e2", [128, 128], F16, kind="ExternalInput")
    cmask_d = nc.dram_tensor("cmask", [128, 2, 128], F16, kind="ExternalInput")
    out_d = nc.dram_tensor("out", [B, RB4, D], F32, kind="ExternalOutput")

    with tile.TileContext(nc) as tc:
        with (
            tc.tile_pool(name="persist", bufs=1) as persist,
            tc.tile_pool(name="xt", bufs=3) as xtp,
            tc.tile_pool(name="ep", bufs=5) as ep,
            tc.tile_pool(name="tail", bufs=2) as tailp,
            tc.tile_pool(name="ctx", bufs=3) as ctxp,
            tc.tile_pool(name="outp", bufs=3) as outp,
            tc.tile_pool(name="ps_s", bufs=2, space="PSUM") as ps_s,
            tc.tile_pool(name="ps_av", bufs=3, space="PSUM") as ps_av,
            tc.tile_pool(name="ps_misc", bufs=1, space="PSUM") as ps_misc,
            tc.tile_pool(name="dram", bufs=1, space="DRAM") as dram,
        ):
            # ---- persistent state ----
            wq_sb = persist.tile([128, KS, 128], F16)
            wk_sb = persist.tile([128, KS, 128], F16)
            wv_sb = persist.tile([128, KS, 128], F16)
            wo_sb = persist.tile([128, KS, D], F16)
            nc.sync.dma_start(wq_sb[:], wq_d.rearrange("(o p) h -> p o h", p=128))
            nc.sync.dma_start(wk_sb[:], wk_d.rearrange("(o p) h -> p o h", p=128))
            nc.sync.dma_start(wv_sb[:], wv_d.rearrange("(o p) h -> p o h", p=128))

            qt_sb = persist.tile([128, BT], F16)  # [2 heads x 64, global t]
            # K^T zero-padded per head: [:, 0, t] rows 0-63 = head0 (rest 0),
            # [:, 1, t] rows 64-127 = head1 (rest 0)
            ktp_sb = persist.tile([128, 2, BT], F16)
            # V layout: [128 keys-in-block, B*NKB blocks, 2*(1+64)]
            #   per head h: cols 0:64 = V_h, col 64 = ones (denominator)
            v_sb = persist.tile([128, B * NKB, 2, DH + 1], F16)
            nc.vector.memset(v_sb[:, :, :, DH : DH + 1], 1.0)

            # per-partition selector scales for the padded K^T casts
            s01 = persist.tile([128, 1], F32)
            s10 = persist.tile([128, 1], F32)
            nc.vector.memset(s01[0:64], 1.0)
            nc.vector.memset(s01[64:128], 0.0)
            nc.vector.memset(s10[0:64], 0.0)
            nc.vector.memset(s10[64:128], 1.0)

            # bias broadcast [128, D] fp32 via PE ones-trick
            ones_col = persist.tile([1, 128], F32)
            nc.vector.memset(ones_col[:], 1.0)
            bo_sb = persist.tile([1, D], F32)
            nc.sync.dma_start(bo_sb[:], bo_d[None, :])
            bias_sb = persist.tile([128, D], F32)
            for nch in range(2):
                bps = ps_misc.tile([128, 512], F32, tag="misc")
                nc.tensor.matmul(
                    bps[:], ones_col[:], bo_sb[:, nch * 512 : (nch + 1) * 512]
                )
                nc.vector.tensor_copy(bias_sb[:, nch * 512 : (nch + 1) * 512], bps[:])

            # padded E2 selector (rows 0-63 <- r2[0], 64-127 <- r2[1]; rows
            # 2-127 of the moving operand are zero)
            e2_sb = persist.tile([128, 128], F16)
            nc.sync.dma_start(e2_sb[:], e2_d[:])
            r2hp = persist.tile([128, QC], F16)
            nc.vector.memset(r2hp[:], 0.0)

            # boundary triangle mask (both heads): cmask[p, h, j] = (j >= p)
            cmask_sb = persist.tile([128, 2, 128], F16)
            nc.sync.dma_start(cmask_sb[:], cmask_d[:])

            # ---- projection emission (as filler items) ----
            def emit_xt_dma(tcn):
                t0 = tcn * TC
                xt = xtp.tile([128, KS, TC], F16, tag="xt", name="xt")
                nc.sync.dma_start(
                    xt[:],
                    x_d[:, t0 : t0 + TC].rearrange("(o p) t -> p o t", p=128),
                )
                return xt

            def emit_q_group(xt, tcn):
                t0 = tcn * TC
                pp = ps_misc.tile([128, TC], F32, tag="misc", name="qp")
                for ks in range(KS):
                    nc.tensor.matmul(
                        pp[:], wq_sb[:, ks, :], xt[:, ks, :],
                        start=(ks == 0), stop=(ks == KS - 1),
                    )
                nc.scalar.copy(qt_sb[:, t0 : t0 + TC], pp[:])

            def emit_k_group(xt, tcn):
                t0 = tcn * TC
                pp = ps_misc.tile([128, TC], F32, tag="misc", name="kp")
                for ks in range(KS):
                    nc.tensor.matmul(
                        pp[:], wk_sb[:, ks, :], xt[:, ks, :],
                        start=(ks == 0), stop=(ks == KS - 1),
                    )
                nc.vector.tensor_scalar_mul(ktp_sb[:, 0, t0 : t0 + TC], pp[:], s01[:])
                nc.vector.tensor_scalar_mul(ktp_sb[:, 1, t0 : t0 + TC], pp[:], s10[:])

            def emit_v_sub(xt, tcn, sub):
                vp = ps_misc.tile([128, 2, DH], F32, tag="misc", name="vp")
                for ks in range(KS):
                    nc.tensor.matmul(
                        vp[:],
                        xt[:, ks, sub * 128 : (sub + 1) * 128],
                        wv_sb[:, ks, :],
                        start=(ks == 0), stop=(ks == KS - 1),
                    )
                kbg = tcn * (TC // 128) + sub
                dst = v_sb[:, kbg, :, 0:DH]  # cols {0..63} u {65..128}
                nc.vector.tensor_copy(dst, vp[:])

            def proj_chunk_items(tcn):
                state = {}

                def first():
                    state["xt"] = emit_xt_dma(tcn)
                    emit_q_group(state["xt"], tcn)

                items = [first]
                items.append(lambda: emit_k_group(state["xt"], tcn))
                for sub in range(TC // 128):
                    items.append(
                        lambda s=sub: emit_v_sub(state["xt"], tcn, s)
                    )
                return items

            # ---- out-projection (as filler items) ----
            cc_ins = [dram.tile([N_CORES, 128, RB4], F16, name=f"cc_in{b}", tag=f"cc_in{b}") for b in range(B - 1)]
            cc_outs = [dram.tile([N_CORES, 128, RB4], F16, name=f"cc_out{b}", tag=f"cc_out{b}") for b in range(B - 1)]
            cc_ins_h = [dram.tile([N_CORES, 128, RB4 // 2], F16, name=f"cc_inh{i}", tag=f"cc_inh{i}") for i in range(2)]
            cc_outs_h = [dram.tile([N_CORES, 128, RB4 // 2], F16, name=f"cc_outh{i}", tag=f"cc_outh{i}") for i in range(2)]
            ao_sbs = []

            def emit_oproj_group(item):
                ob, oao, mb, nch = item
                t_in_ao = (mb * 128) % oao.shape[2]
                op = ps_misc.tile([128, 512], F32, tag="misc", name="op")
                for r in range(KS):
                    nc.tensor.matmul(
                        op[:],
                        oao[:, r, t_in_ao : t_in_ao + 128],
                        wo_sb[:, r, nch * 512 : (nch + 1) * 512],
                        start=(r == 0), stop=(r == KS - 1),
                    )
                osb = outp.tile([128, 512], F32, tag="osb", name="osb")
                nc.vector.tensor_tensor(
                    osb[:], op[:], bias_sb[:, nch * 512 : (nch + 1) * 512],
                    mybir.AluOpType.add,
                )
                nc.sync.dma_start(
                    out_d[ob, mb * 128 : (mb + 1) * 128,
                          nch * 512 : (nch + 1) * 512],
                    osb[:],
                )

            # ---- filler queue: keeps the PE fed between attention blocks.
            # Items carry a (b, qc) gate: not poppable before that position
            # (so an out-proj group never stalls the in-order PE behind its
            # AllToAll). cur_pos is updated by the attention loop. ----
            filler = []
            cur_pos = [0, 0]

            def pop_filler(n=1):
                popped = 0
                i = 0
                while popped < n and i < len(filler):
                    gate, fn = filler[i]
                    if gate <= (cur_pos[0], cur_pos[1]):
                        filler.pop(i)
                        fn()
                        popped += 1
                    else:
                        i += 1

            # batch 0 projections (+ chunks 4-5) emitted up front
            for tcn in range(6):
                for it in proj_chunk_items(tcn):
                    it()
            nc.sync.dma_start(wo_sb[:], wo_d.rearrange("(r p) n -> p r n", p=128))
            for tcn in range(6, NTC):
                for it in proj_chunk_items(tcn):
                    filler.append(((0, 0), it))

            def emit_half_a2a(half):
                nc.gpsimd.collective_compute(
                    "AllToAll",
                    mybir.AluOpType.bypass,
                    replica_groups=[list(range(N_CORES))],
                    ins=[cc_ins_h[half][:]],
                    outs=[cc_outs_h[half][:]],
                )
                RBH = RB4 // 2
                ao_sb = persist.tile([128, KS, RBH], F16, name=f"aoh{half}", tag=f"aoh{half}")
                ao_sbs.append(ao_sb)
                nc.sync.dma_start(ao_sb[:], cc_outs_h[half].rearrange("r p t -> p r t"))
                gate = (B - 1, 3) if half == 0 else (B, 0)
                for nch in range(2):
                    filler.append(
                        (gate,
                         lambda a=ao_sb, h=half, n=nch: emit_oproj_group((B - 1, a, h, n)))
                    )

            # ---- attention ----
            pending_tail = []

            def emit_qc_tail_head(av0, av1):
                # DVE-only reciprocal chain, emitted right at qc end (no PE
                # involvement); reciprocal_approx_fast needs partition-0 ops
                d2a = tailp.tile([1, QC], F32, tag="d2a")
                d2b = tailp.tile([1, QC], F32, tag="d2b")
                nc.vector.tensor_copy(d2a[:], av0[DH : DH + 1, :])
                nc.vector.tensor_copy(d2b[:], av1[DH : DH + 1, :])
                r2a = tailp.tile([1, QC], F32, tag="r2a")
                r2b = tailp.tile([1, QC], F32, tag="r2b")
                nc.vector.reciprocal_approx_fast(r2a[:], d2a[:])
                nc.vector.reciprocal_approx_fast(r2b[:], d2b[:])
                nc.vector.tensor_copy(r2hp[0:1, :], r2a[:])
                nc.vector.tensor_copy(r2hp[64:65, :], r2b[:])

            def emit_qc_tail(b, qc, av0, av1):
                rb = ps_misc.tile([128, QC], F32, tag="misc", name="rb")
                nc.tensor.matmul(rb[:], e2_sb[:], r2hp[:])
                rb_sb = tailp.tile([128, QC], F16, tag="rbs")
                nc.vector.tensor_copy(rb_sb[:], rb[:])
                ctx2 = ctxp.tile([128, QC], F16, tag="ctx")
                nc.vector.tensor_tensor(
                    ctx2[0:64, :], av0[0:DH, :], rb_sb[0:64, :],
                    mybir.AluOpType.mult,
                )
                nc.vector.tensor_tensor(
                    ctx2[64:128, :], av1[0:DH, :], rb_sb[64:128, :],
                    mybir.AluOpType.mult,
                )
                if b < B - 1:
                    s0 = qc * QC // RB4
                    nc.sync.dma_start(
                        cc_ins[b][s0 : s0 + QC // RB4].rearrange("s p f -> p s f"),
                        ctx2[:].rearrange("p (s f) -> p s f", s=QC // RB4),
                    )
                else:
                    half, RBH = qc // 2, RB4 // 2
                    s0 = (qc % 2) * QC // RBH
                    nc.sync.dma_start(
                        cc_ins_h[half][s0 : s0 + QC // RBH].rearrange("s p f -> p s f"),
                        ctx2[:].rearrange("p (s f) -> p s f", s=QC // RBH),
                    )

            def emit_batch_a2a(b):
                nc.gpsimd.collective_compute(
                    "AllToAll",
                    mybir.AluOpType.bypass,
                    replica_groups=[list(range(N_CORES))],
                    ins=[cc_ins[b][:]],
                    outs=[cc_outs[b][:]],
                )
                ao_sb = persist.tile([128, KS, RB4], F16, name=f"ao{b}", tag=f"ao{b}")
                ao_sbs.append(ao_sb)
                nc.sync.dma_start(ao_sb[:], cc_outs[b].rearrange("r p t -> p r t"))
                for mb in range(RB4 // 128):
                    for nch in range(2):
                        filler.append(
                            ((b + 1, 2),
                             lambda a=ao_sb, m=mb, n=nch, bb=b: emit_oproj_group(
                                 (bb, a, m, n)
                             ))
                        )

            for b in range(B):
                for qc in range(NQC):
                    cur_pos[0], cur_pos[1] = b, qc
                    q0 = b * T + qc * QC
                    nkb = 4 * qc + 4

                    def emit_se(kb):
                        # scores + merged exp (+ triangle mask) for block kb
                        k0 = b * T + kb * KB
                        diag_i = kb - 4 * qc  # >= 0 for diagonal blocks
                        lo = max(0, diag_i) * 128  # live query range start
                        sp = ps_s.tile([128, 2, QC], F32, tag="s", name="sp")
                        for h in (0, 1):
                            nc.tensor.matmul(
                                sp[:, h, lo:QC],
                                ktp_sb[:, h, k0 : k0 + KB],
                                qt_sb[:, q0 + lo : q0 + QC],
                            )
                        e = ep.tile([128, 2, QC], F16, tag="e", name="e")
                        nc.scalar.activation(
                            e[:, :, lo:QC], sp[:, :, lo:QC],
                            mybir.ActivationFunctionType.Exp,
                            scale=0.125,
                        )
                        if diag_i >= 0:
                            nc.gpsimd.tensor_tensor(
                                e[:, :, lo : lo + 128],
                                e[:, :, lo : lo + 128],
                                cmask_sb[:],
                                mybir.AluOpType.mult,
                            )
                        return e, lo

                    # warm the S/exp pipeline, then flush the previous
                    # q-chunk's deferred tail (reads its av psum) BEFORE
                    # allocating this q-chunk's av tiles, then launch any
                    # collective whose inputs that tail produced
                    e_q = [emit_se(0)]
                    next_emit = 1
                    if nkb > 1:
                        e_q.append(emit_se(1))
                        next_emit = 2
                    while pending_tail:
                        pending_tail.pop(0)()
                    if qc == 0 and b > 0:
                        emit_batch_a2a(b - 1)
                    if b == B - 1 and qc == 2:
                        emit_half_a2a(0)

                    av0_full = ps_av.tile([128, QC], F32, tag="av", name="av0")
                    av1_full = ps_av.tile([128, QC], F32, tag="av", name="av1")
                    av0 = av0_full[: DH + 1]
                    av1 = av1_full[: DH + 1]

                    for kb in range(nkb):
                        kbg = b * NKB + kb
                        first, last = kb == 0, kb == nkb - 1
                        e_cur, lo = e_q.pop(0)
                        if kb > 0 and next_emit < nkb:
                            e_q.append(emit_se(next_emit))
                            next_emit += 1
                        nc.tensor.matmul(
                            av0[:, lo:QC], v_sb[:, kbg, 0, :], e_cur[:, 0, lo:QC],
                            start=first, stop=last,
                        )
                        if kb == 0 and next_emit < nkb:
                            # delay head1's first accumulation so the deferred
                            # division chain on DVE can release its av bank
                            e_q.append(emit_se(next_emit))
                            next_emit += 1
                        nc.tensor.matmul(
                            av1[:, lo:QC], v_sb[:, kbg, 1, :], e_cur[:, 1, lo:QC],
                            start=first, stop=last,
                        )
                        if kb % 2 == 1:
                            pop_filler(1)
                    emit_qc_tail_head(av0, av1)
                    pending_tail.append(
                        lambda b_=b, qc_=qc, a0=av0, a1=av1: emit_qc_tail(
                            b_, qc_, a0, a1
                        )
                    )

            while pending_tail:
                pending_tail.pop(0)()
            emit_half_a2a(1)
            cur_pos[0], cur_pos[1] = B, 0
            while filler:
                pop_filler(1)

    nc.compile()
    return nc


def _get_nc():
    if "nc" not in _CACHE:
        _CACHE["nc"] = _build()
    return _CACHE["nc"]


def prepare_in_maps(x, Wq, Wk, Wv, Wo, bo):
    x16 = np.ascontiguousarray(np.asarray(x, dtype=np.float32).reshape(BT, D).T).astype(np.float16)
    wo16 = np.asarray(Wo, dtype=np.float32).astype(np.float16)
    bo32 = np.ascontiguousarray(np.asarray(bo, dtype=np.float32))
    e2 = np.zeros((128, 128), dtype=np.float16)
    e2[0, 0:64] = 1.0
    e2[64, 64:128] = 1.0
    p = np.arange(128)[:, None]
    j = np.arange(128)[None, :]
    cmask = np.broadcast_to((j >= p).astype(np.float16)[:, None, :], (128, 2, 128))
    cmask = np.ascontiguousarray(cmask)
    in_maps = []
    for c in range(N_CORES):
        cs = slice(128 * c, 128 * (c + 1))
        in_maps.append(
            {
                "x": x16,
                "wq": np.ascontiguousarray(np.asarray(Wq, np.float32)[:, cs]).astype(np.float16),
                "wk": np.ascontiguousarray(np.asarray(Wk, np.float32)[:, cs]).astype(np.float16),
                "wv": np.ascontiguousarray(np.asarray(Wv, np.float32)[:, cs]).astype(np.float16),
                "wo": wo16,
                "bo": bo32,
                "e2": e2,
                "cmask": cmask,
            }
        )
    return in_maps


def kernel(x, Wq, Wk, Wv, Wo, bo, _trace=False):
    nc = _get_nc()
    in_maps = prepare_in_maps(x, Wq, Wk, Wv, Wo, bo)
    res = bass_utils.run_bass_kernel_spmd(
        nc, in_maps, list(range(N_CORES)), trace=_trace
    )
    if _trace:
        _CACHE["last_results"] = res
    out = np.empty((B, T, D), dtype=np.float32)
    rb4 = ROWS // B
    rbh = rb4 // 2
    for c in range(N_CORES):
        oc = res.results[c]["out"]  # [B, 256, D]
        for b in range(B - 1):
            out[b, rb4 * c : rb4 * (c + 1), :] = oc[b]
        # last batch was exchanged as two half-batch A2As with 128-row shards
        out[B - 1, rbh * c : rbh * (c + 1), :] = oc[B - 1, 0:rbh]
        out[B - 1, T // 2 + rbh * c : T // 2 + rbh * (c + 1), :] = oc[B - 1, rbh:]
    return out


# revision 13
# speedup vs baseline: 1.1724x; 1.0542x over previous
"""Trainium2 Bass kernel for causal multi-head attention.

Shapes (hardcoded): B=4, T=2048, D=1024, H=16, Dh=64, fp32 I/O.

Strategy (8 NeuronCores, tensor-parallel over heads):
  - Each core c owns heads (2c, 2c+1): computes Q^T/K^T/V projections for its
    128 head-dims over the whole [B*T, D] input (contracting D on the PE),
    then causal flash-style attention in "scores-transposed" orientation
    (S^T[k, q] blocks) so softmax needs no on-chip transposes:
      * exp on ScalarE, one merged instruction per key-block covering both
        heads ([128, 2, width] over a 2-bank PSUM group)
      * causal handling at 128-column granularity: S matmuls, exps and AV
        matmuls of the 4 diagonal blocks of each q-chunk are narrowed to the
        live query range; only the 128x128 boundary triangle gets a mask
      * denominator via a leading ones-column in the V stationary operand
        (row 0 of the AV psum = sum of exp weights)
      * division folded into the PSUM->SBUF cast against a PE-broadcast
        reciprocal
  - K^T is stored zero-padded per head ([128, 2, BT]) so every matmul in the
    kernel runs in the PE's 128x128 tile mode (no tiling-mode switches).
  - Projection / out-projection matmul groups are emitted as *filler* between
    attention blocks so the PE never idles (sustains the 2.4 GHz p-state).
  - An on-device AllToAll re-shards ctx^T from head-sharded to row-sharded,
    then each core computes out rows = ctx @ Wo + bo.

All matmul operands are fp16; accumulation is fp32 in PSUM.
"""

import sys

sys.path.insert(0, "/opt/trn_rl_repo")

import numpy as np

import concourse.bass as bass
import concourse.mybir as mybir
import concourse.tile as tile
from concourse import bacc
from concourse import bass_utils

N_CORES = 8
B, T, D, H, DH = 4, 2048, 1024, 16, 64
BT = B * T  # 8192
KS = D // 128  # 8 contraction subtiles
TC = 512  # t-chunk for projections
NTC = BT // TC  # 16
QC = 512  # query chunk in attention
NQC = T // QC  # 4 per batch
KB = 128  # key block
NKB = T // KB  # 16 per batch
ROWS = BT // N_CORES  # 1024 out rows per core
RB4 = ROWS // B  # 256 out rows per core per batch

F16 = mybir.dt.float16
F32 = mybir.dt.float32

_CACHE = {}


def _build():
    nc = bacc.Bacc("TRN2", target_bir_lowering=False, num_devices=N_CORES)

    x_d = nc.dram_tensor("x", [D, BT], F16, kind="ExternalInput")  # pre-transposed
    wq_d = nc.dram_tensor("wq", [D, 128], F16, kind="ExternalInput")
    wk_d = nc.dram_tensor("wk", [D, 128], F16, kind="ExternalInput")
    wv_d = nc.dram_tensor("wv", [D, 128], F16, kind="ExternalInput")
    wo_d = nc.dram_tensor("wo", [D, D], F16, kind="ExternalInput")
    bo_d = nc.dram_tensor("bo", [D], F32, kind="ExternalInput")
    e2_d = nc.dram_tensor("e2", [128, 128], F16, kind="ExternalInput")
    cmask_d = nc.dram_tensor("cmask", [128, 2, 128], F16, kind="ExternalInput")
    out_d = nc.dram_tensor("out", [B, RB4, D], F32, kind="ExternalOutput")

    with tile.TileContext(nc) as tc:
        with (
            tc.tile_pool(name="persist", bufs=1) as persist,
            tc.tile_pool(name="xt", bufs=3) as xtp,
            tc.tile_pool(name="ep", bufs=5) as ep,
            tc.tile_pool(name="tail", bufs=2) as tailp,
            tc.tile_pool(name="ctx", bufs=3) as ctxp,
            tc.tile_pool(name="outp", bufs=3) as outp,
            tc.tile_pool(name="ps_s", bufs=2, space="PSUM") as ps_s,
            tc.tile_pool(name="ps_av", bufs=2, space="PSUM") as ps_av,
            tc.tile_pool(name="ps_misc", bufs=2, space="PSUM") as ps_misc,
            tc.tile_pool(name="dram", bufs=1, space="DRAM") as dram,
        ):
            # ---- persistent state ----
            wq_sb = persist.tile([128, KS, 128], F16)
            wk_sb = persist.tile([128, KS, 128], F16)
            wv_sb = persist.tile([128, KS, 128], F16)
            wo_sb = persist.tile([128, KS, D], F16)
            nc.sync.dma_start(wq_sb[:], wq_d.rearrange("(o p) h -> p o h", p=128))

            qt_sb = persist.tile([128, BT], F16)  # [2 heads x 64, global t]
            # K^T zero-padded per head: [:, 0, t] rows 0-63 = head0 (rest 0),
            # [:, 1, t] rows 64-127 = head1 (rest 0)
            ktp_sb = persist.tile([128, 2, BT], F16)
            # V layout: [128 keys-in-block, B*NKB blocks, 2*(1+64)]
            #   per head h: cols 0:64 = V_h, col 64 = ones (denominator)
            v_sb = persist.tile([128, B * NKB, 2, DH + 1], F16)
            nc.vector.memset(v_sb[:, :, :, DH : DH + 1], 1.0)

            # per-partition selector scales for the padded K^T casts
            s01 = persist.tile([128, 1], F32)
            s10 = persist.tile([128, 1], F32)
            nc.vector.memset(s01[0:64], 1.0)
            nc.vector.memset(s01[64:128], 0.0)
            nc.vector.memset(s10[0:64], 0.0)
            nc.vector.memset(s10[64:128], 1.0)

            # bias broadcast [128, D] fp32 via PE ones-trick
            ones_col = persist.tile([1, 128], F32)
            nc.vector.memset(ones_col[:], 1.0)
            bo_sb = persist.tile([1, D], F32)
            nc.sync.dma_start(bo_sb[:], bo_d[None, :])
            bias_sb = persist.tile([128, D], F32)
            for nch in range(2):
                bps = ps_misc.tile([128, 512], F32, tag="misc")
                nc.tensor.matmul(
                    bps[:], ones_col[:], bo_sb[:, nch * 512 : (nch + 1) * 512]
                )
                nc.vector.tensor_copy(bias_sb[:, nch * 512 : (nch + 1) * 512], bps[:])

            # padded E2 selector (rows 0-63 <- r2[0], 64-127 <- r2[1]; rows
            # 2-127 of the moving operand are zero)
            e2_sb = persist.tile([128, 128], F16)
            nc.sync.dma_start(e2_sb[:], e2_d[:])
            r2hp = persist.tile([128, QC], F16)
            nc.vector.memset(r2hp[:], 0.0)

            # boundary triangle mask (both heads): cmask[p, h, j] = (j >= p)
            cmask_sb = persist.tile([128, 2, 128], F16)
            nc.sync.dma_start(cmask_sb[:], cmask_d[:])

            # ---- projection emission (as filler items) ----
            def emit_xt_dma(tcn):
                t0 = tcn * TC
                xt = xtp.tile([128, KS, TC], F16, tag="xt", name="xt")
                nc.sync.dma_start(
                    xt[:],
                    x_d[:, t0 : t0 + TC].rearrange("(o p) t -> p o t", p=128),
                )
                return xt

            def emit_q_group(xt, tcn):
                t0 = tcn * TC
                pp = ps_misc.tile([128, TC], F32, tag="misc", name="qp")
                for ks in range(KS):
                    nc.tensor.matmul(
                        pp[:], wq_sb[:, ks, :], xt[:, ks, :],
                        start=(ks == 0), stop=(ks == KS - 1),
                    )
                nc.scalar.copy(qt_sb[:, t0 : t0 + TC], pp[:])

            def emit_k_group(xt, tcn):
                t0 = tcn * TC
                pp = ps_misc.tile([128, TC], F32, tag="misc", name="kp")
                for ks in range(KS):
                    nc.tensor.matmul(
                        pp[:], wk_sb[:, ks, :], xt[:, ks, :],
                        start=(ks == 0), stop=(ks == KS - 1),
                    )
                nc.scalar.mul(ktp_sb[:, 0, t0 : t0 + TC], pp[:], s01[:])
                nc.vector.tensor_scalar_mul(ktp_sb[:, 1, t0 : t0 + TC], pp[:], s10[:])

            def emit_v_sub(xt, tcn, sub):
                vp = ps_misc.tile([128, 2, DH], F32, tag="misc", name="vp")
                for ks in range(KS):
                    nc.tensor.matmul(
                        vp[:],
                        xt[:, ks, sub * 128 : (sub + 1) * 128],
                        wv_sb[:, ks, :],
                        start=(ks == 0), stop=(ks == KS - 1),
                    )
                kbg = tcn * (TC // 128) + sub
                dst = v_sb[:, kbg, :, 0:DH]  # cols {0..63} u {65..128}
                nc.vector.tensor_copy(dst, vp[:])

            def proj_chunk_items(tcn):
                state = {}

                def first():
                    state["xt"] = emit_xt_dma(tcn)
                    emit_q_group(state["xt"], tcn)

                items = [first]
                items.append(lambda: emit_k_group(state["xt"], tcn))
                for sub in range(TC // 128):
                    items.append(
                        lambda s=sub: emit_v_sub(state["xt"], tcn, s)
                    )
                return items

            # ---- out-projection (as filler items) ----
            cc_ins = [dram.tile([N_CORES, 128, RB4], F16, name=f"cc_in{b}", tag=f"cc_in{b}") for b in range(B - 1)]
            cc_outs = [dram.tile([N_CORES, 128, RB4], F16, name=f"cc_out{b}", tag=f"cc_out{b}") for b in range(B - 1)]
            cc_ins_h = [dram.tile([N_CORES, 128, RB4 // 2], F16, name=f"cc_inh{i}", tag=f"cc_inh{i}") for i in range(2)]
            cc_outs_h = [dram.tile([N_CORES, 128, RB4 // 2], F16, name=f"cc_outh{i}", tag=f"cc_outh{i}") for i in range(2)]
            ao_sbs = []

            def emit_oproj_group(item):
                ob, oao, mb, nch = item
                t_in_ao = (mb * 128) % oao.shape[2]
                op = ps_misc.tile([128, 512], F32, tag="misc", name="op")
                for r in range(KS):
                    nc.tensor.matmul(
                        op[:],
                        oao[:, r, t_in_ao : t_in_ao + 128],
                        wo_sb[:, r, nch * 512 : (nch + 1) * 512],
                        start=(r == 0), stop=(r == KS - 1),
                    )
                osb = outp.tile([128, 512], F32, tag="osb", name="osb")
                nc.vector.tensor_tensor(
                    osb[:], op[:], bias_sb[:, nch * 512 : (nch + 1) * 512],
                    mybir.AluOpType.add,
                )
                nc.sync.dma_start(
                    out_d[ob, mb * 128 : (mb + 1) * 128,
                          nch * 512 : (nch + 1) * 512],
                    osb[:],
                )

            # ---- filler queue: keeps the PE fed between attention blocks.
            # Items carry a (b, qc) gate: not poppable before that position
            # (so an out-proj group never stalls the in-order PE behind its
            # AllToAll). cur_pos is updated by the attention loop. ----
            filler = []
            cur_pos = [0, 0]

            def pop_filler(n=1):
                popped = 0
                i = 0
                while popped < n and i < len(filler):
                    gate, fn = filler[i]
                    if gate <= (cur_pos[0], cur_pos[1]):
                        filler.pop(i)
                        fn()
                        popped += 1
                    else:
                        i += 1

            # batch 0 projections (+ chunks 4-5) emitted up front; first
            # x chunks' DMAs interleaved with the remaining weight DMAs so
            # the PE can start as soon as wq + xt0 land
            xt0 = emit_xt_dma(0)
            nc.sync.dma_start(wk_sb[:], wk_d.rearrange("(o p) h -> p o h", p=128))
            xt1 = emit_xt_dma(1)
            nc.sync.dma_start(wv_sb[:], wv_d.rearrange("(o p) h -> p o h", p=128))
            emit_q_group(xt0, 0)
            emit_k_group(xt0, 0)
            for sub in range(TC // 128):
                emit_v_sub(xt0, 0, sub)
            emit_q_group(xt1, 1)
            emit_k_group(xt1, 1)
            for sub in range(TC // 128):
                emit_v_sub(xt1, 1, sub)
            for tcn in range(2, 6):
                for it in proj_chunk_items(tcn):
                    it()
            nc.sync.dma_start(wo_sb[:], wo_d.rearrange("(r p) n -> p r n", p=128))
            for tcn in range(6, NTC):
                for it in proj_chunk_items(tcn):
                    filler.append(((0, 0), it))

            def emit_half_a2a(half):
                nc.gpsimd.collective_compute(
                    "AllToAll",
                    mybir.AluOpType.bypass,
                    replica_groups=[list(range(N_CORES))],
                    ins=[cc_ins_h[half][:]],
                    outs=[cc_outs_h[half][:]],
                )
                RBH = RB4 // 2
                ao_sb = persist.tile([128, KS, RBH], F16, name=f"aoh{half}", tag=f"aoh{half}")
                ao_sbs.append(ao_sb)
                nc.sync.dma_start(ao_sb[:], cc_outs_h[half].rearrange("r p t -> p r t"))
                gate = (B - 1, 3) if half == 0 else (B, 0)
                for nch in range(2):
                    filler.append(
                        (gate,
                         lambda a=ao_sb, h=half, n=nch: emit_oproj_group((B - 1, a, h, n)))
                    )

            # ---- attention ----
            pending_tail = []

            def emit_qc_tail_head(av0, av1):
                # DVE-only reciprocal chain, emitted right at qc end (no PE
                # involvement); reciprocal_approx_fast needs partition-0 ops
                d2a = tailp.tile([1, QC], F32, tag="d2a")
                d2b = tailp.tile([1, QC], F32, tag="d2b")
                nc.vector.tensor_copy(d2a[:], av0[DH : DH + 1, :])
                nc.vector.tensor_copy(d2b[:], av1[DH : DH + 1, :])
                r2a = tailp.tile([1, QC], F32, tag="r2a")
                r2b = tailp.tile([1, QC], F32, tag="r2b")
                nc.vector.reciprocal_approx_fast(r2a[:], d2a[:])
                nc.vector.reciprocal_approx_fast(r2b[:], d2b[:])
                nc.vector.tensor_copy(r2hp[0:1, :], r2a[:])
                nc.vector.tensor_copy(r2hp[64:65, :], r2b[:])

            def emit_qc_tail(b, qc, av0, av1):
                rb = ps_misc.tile([128, QC], F32, tag="misc", name="rb")
                nc.tensor.matmul(rb[:], e2_sb[:], r2hp[:])
                rb_sb = tailp.tile([128, QC], F16, tag="rbs")
                nc.vector.tensor_copy(rb_sb[:], rb[:])
                ctx2 = ctxp.tile([128, QC], F16, tag="ctx")
                nc.vector.tensor_tensor(
                    ctx2[0:64, :], av0[0:DH, :], rb_sb[0:64, :],
                    mybir.AluOpType.mult,
                )
                nc.vector.tensor_tensor(
                    ctx2[64:128, :], av1[0:DH, :], rb_sb[64:128, :],
                    mybir.AluOpType.mult,
                )
                if b < B - 1:
                    s0 = qc * QC // RB4
                    nc.sync.dma_start(
                        cc_ins[b][s0 : s0 + QC // RB4].rearrange("s p f -> p s f"),
                        ctx2[:].rearrange("p (s f) -> p s f", s=QC // RB4),
                    )
                else:
                    half, RBH = qc // 2, RB4 // 2
                    s0 = (qc % 2) * QC // RBH
                    nc.sync.dma_start(
                        cc_ins_h[half][s0 : s0 + QC // RBH].rearrange("s p f -> p s f"),
                        ctx2[:].rearrange("p (s f) -> p s f", s=QC // RBH),
                    )

            def emit_batch_a2a(b):
                nc.gpsimd.collective_compute(
                    "AllToAll",
                    mybir.AluOpType.bypass,
                    replica_groups=[list(range(N_CORES))],
                    ins=[cc_ins[b][:]],
                    outs=[cc_outs[b][:]],
                )
                ao_sb = persist.tile([128, KS, RB4], F16, name=f"ao{b}", tag=f"ao{b}")
                ao_sbs.append(ao_sb)
                nc.sync.dma_start(ao_sb[:], cc_outs[b].rearrange("r p t -> p r t"))
                for mb in range(RB4 // 128):
                    for nch in range(2):
                        filler.append(
                            ((b + 1, 2),
                             lambda a=ao_sb, m=mb, n=nch, bb=b: emit_oproj_group(
                                 (bb, a, m, n)
                             ))
                        )

            for b in range(B):
                for qc in range(NQC):
                    cur_pos[0], cur_pos[1] = b, qc
                    q0 = b * T + qc * QC
                    nkb = 4 * qc + 4

                    def emit_se(kb):
                        # scores + merged exp (+ triangle mask) for block kb
                        k0 = b * T + kb * KB
                        diag_i = kb - 4 * qc  # >= 0 for diagonal blocks
                        lo = max(0, diag_i) * 128  # live query range start
                        sp = ps_s.tile([128, 2, QC], F32, tag="s", name="sp")
                        for h in (0, 1):
                            nc.tensor.matmul(
                                sp[:, h, lo:QC],
                                ktp_sb[:, h, k0 : k0 + KB],
                                qt_sb[:, q0 + lo : q0 + QC],
                            )
                        e = ep.tile([128, 2, QC], F16, tag="e", name="e")
                        nc.scalar.activation(
                            e[:, :, lo:QC], sp[:, :, lo:QC],
                            mybir.ActivationFunctionType.Exp,
                            scale=0.125,
                        )
                        if diag_i >= 0:
                            nc.gpsimd.tensor_tensor(
                                e[:, :, lo : lo + 128],
                                e[:, :, lo : lo + 128],
                                cmask_sb[:],
                                mybir.AluOpType.mult,
                            )
                        return e, lo

                    # warm the S/exp pipeline, then flush the previous
                    # q-chunk's deferred tail (reads its av psum) BEFORE
                    # allocating this q-chunk's av tiles, then launch any
                    # collective whose inputs that tail produced
                    e_q = [emit_se(0)]
                    next_emit = 1
                    if nkb > 1:
                        e_q.append(emit_se(1))
                        next_emit = 2
                    while pending_tail:
                        pending_tail.pop(0)()
                    if qc == 0 and b > 0:
                        emit_batch_a2a(b - 1)
                    if b == B - 1 and qc == 2:
                        emit_half_a2a(0)

                    av0_full = ps_av.tile([128, QC], F32, tag="av", name="av0")
                    av1_full = ps_av.tile([128, QC], F32, tag="av", name="av1")
                    av0 = av0_full[: DH + 1]
                    av1 = av1_full[: DH + 1]

                    for kb in range(nkb):
                        kbg = b * NKB + kb
                        first, last = kb == 0, kb == nkb - 1
                        e_cur, lo = e_q.pop(0)
                        if kb > 0 and next_emit < nkb:
                            e_q.append(emit_se(next_emit))
                            next_emit += 1
                        nc.tensor.matmul(
                            av0[:, lo:QC], v_sb[:, kbg, 0, :], e_cur[:, 0, lo:QC],
                            start=first, stop=last,
                        )
                        if kb == 0 and next_emit < nkb:
                            # delay head1's first accumulation so the deferred
                            # division chain on DVE can release its av bank
                            e_q.append(emit_se(next_emit))
                            next_emit += 1
                        nc.tensor.matmul(
                            av1[:, lo:QC], v_sb[:, kbg, 1, :], e_cur[:, 1, lo:QC],
                            start=first, stop=last,
                        )
                        if kb % 2 == 1:
                            pop_filler(1)
                    emit_qc_tail_head(av0, av1)
                    pending_tail.append(
                        lambda b_=b, qc_=qc, a0=av0, a1=av1: emit_qc_tail(
                            b_, qc_, a0, a1
                        )
                    )

            while pending_tail:
                pending_tail.pop(0)()
            emit_half_a2a(1)
            cur_pos[0], cur_pos[1] = B, 0
            while filler:
                pop_filler(1)

    nc.compile()
    return nc


def _get_nc():
    if "nc" not in _CACHE:
        _CACHE["nc"] = _build()
    return _CACHE["nc"]


def prepare_in_maps(x, Wq, Wk, Wv, Wo, bo):
    x16 = np.ascontiguousarray(np.asarray(x, dtype=np.float32).reshape(BT, D).T).astype(np.float16)
    wo16 = np.asarray(Wo, dtype=np.float32).astype(np.float16)
    bo32 = np.ascontiguousarray(np.asarray(bo, dtype=np.float32))
    e2 = np.zeros((128, 128), dtype=np.float16)
    e2[0, 0:64] = 1.0
    e2[64, 64:128] = 1.0
    p = np.arange(128)[:, None]
    j = np.arange(128)[None, :]
    cmask = np.broadcast_to((j >= p).astype(np.float16)[:, None, :], (128, 2, 128))
    cmask = np.ascontiguousarray(cmask)
    in_maps = []
    for c in range(N_CORES):
        cs = slice(128 * c, 128 * (c + 1))
        in_maps.append(
            {
                "x": x16,
                "wq": np.ascontiguousarray(np.asarray(Wq, np.float32)[:, cs]).astype(np.float16),
                "wk": np.ascontiguousarray(np.asarray(Wk, np.float32)[:, cs]).astype(np.float16),
                "wv": np.ascontiguousarray(np.asarray(Wv, np.float32)[:, cs]).astype(np.float16),
                "wo": wo16,
                "bo": bo32,
                "e2": e2,
                "cmask": cmask,
            }
        )
    return in_maps


def kernel(x, Wq, Wk, Wv, Wo, bo, _trace=False):
    nc = _get_nc()
    in_maps = prepare_in_maps(x, Wq, Wk, Wv, Wo, bo)
    res = bass_utils.run_bass_kernel_spmd(
        nc, in_maps, list(range(N_CORES)), trace=_trace
    )
    if _trace:
        _CACHE["last_results"] = res
    out = np.empty((B, T, D), dtype=np.float32)
    rb4 = ROWS // B
    rbh = rb4 // 2
    for c in range(N_CORES):
        oc = res.results[c]["out"]  # [B, 256, D]
        for b in range(B - 1):
            out[b, rb4 * c : rb4 * (c + 1), :] = oc[b]
        # last batch was exchanged as two half-batch A2As with 128-row shards
        out[B - 1, rbh * c : rbh * (c + 1), :] = oc[B - 1, 0:rbh]
        out[B - 1, T // 2 + rbh * c : T // 2 + rbh * (c + 1), :] = oc[B - 1, rbh:]
    return out


# revision 14
# speedup vs baseline: 1.2031x; 1.0262x over previous
"""Trainium2 Bass kernel for causal multi-head attention.

Shapes (hardcoded): B=4, T=2048, D=1024, H=16, Dh=64, fp32 I/O.

Strategy (8 NeuronCores, tensor-parallel over heads):
  - Each core c owns heads (2c, 2c+1): computes Q^T/K^T/V projections for its
    128 head-dims over the whole [B*T, D] input (contracting D on the PE),
    then causal flash-style attention in "scores-transposed" orientation
    (S^T[k, q] blocks) so softmax needs no on-chip transposes:
      * exp on ScalarE, one merged instruction per key-block covering both
        heads ([128, 2, width] over a 2-bank PSUM group)
      * causal handling at 128-column granularity: S matmuls, exps and AV
        matmuls of the 4 diagonal blocks of each q-chunk are narrowed to the
        live query range; only the 128x128 boundary triangle gets a mask
      * denominator via a leading ones-column in the V stationary operand
        (row 0 of the AV psum = sum of exp weights)
      * division folded into the PSUM->SBUF cast against a PE-broadcast
        reciprocal
  - K^T is stored zero-padded per head ([128, 2, BT]) so every matmul in the
    kernel runs in the PE's 128x128 tile mode (no tiling-mode switches).
  - Projection / out-projection matmul groups are emitted as *filler* between
    attention blocks so the PE never idles (sustains the 2.4 GHz p-state).
  - An on-device AllToAll re-shards ctx^T from head-sharded to row-sharded,
    then each core computes out rows = ctx @ Wo + bo.

All matmul operands are fp16; accumulation is fp32 in PSUM.
"""

import sys

sys.path.insert(0, "/opt/trn_rl_repo")

import numpy as np

import concourse.bass as bass
import concourse.mybir as mybir
import concourse.tile as tile
from concourse import bacc
from concourse import bass_utils

N_CORES = 8
B, T, D, H, DH = 4, 2048, 1024, 16, 64
BT = B * T  # 8192
KS = D // 128  # 8 contraction subtiles
TC = 512  # t-chunk for projections
NTC = BT // TC  # 16
QC = 512  # query chunk in attention
NQC = T // QC  # 4 per batch
KB = 128  # key block
NKB = T // KB  # 16 per batch
ROWS = BT // N_CORES  # 1024 out rows per core
RB4 = ROWS // B  # 256 out rows per core per batch

F16 = mybir.dt.float16
F32 = mybir.dt.float32

_CACHE = {}


def _build():
    nc = bacc.Bacc("TRN2", target_bir_lowering=False, num_devices=N_CORES)

    x_d = nc.dram_tensor("x", [D, BT], F16, kind="ExternalInput")  # pre-transposed
    wq_d = nc.dram_tensor("wq", [D, 128], F16, kind="ExternalInput")
    wk_d = nc.dram_tensor("wk", [D, 128], F16, kind="ExternalInput")
    wv_d = nc.dram_tensor("wv", [D, 128], F16, kind="ExternalInput")
    wo_d = nc.dram_tensor("wo", [D, D], F16, kind="ExternalInput")
    bo_d = nc.dram_tensor("bo", [D], F32, kind="ExternalInput")
    e2_d = nc.dram_tensor("e2", [128, 128], F16, kind="ExternalInput")
    cmask_d = nc.dram_tensor("cmask", [128, 2, 128], F16, kind="ExternalInput")
    out_d = nc.dram_tensor("out", [B, RB4, D], F32, kind="ExternalOutput")

    with tile.TileContext(nc) as tc:
        with (
            tc.tile_pool(name="persist", bufs=1) as persist,
            tc.tile_pool(name="xt", bufs=4) as xtp,
            tc.tile_pool(name="ep", bufs=6) as ep,
            tc.tile_pool(name="tail", bufs=2) as tailp,
            tc.tile_pool(name="ctx", bufs=3) as ctxp,
            tc.tile_pool(name="outp", bufs=3) as outp,
            tc.tile_pool(name="ps_s", bufs=2, space="PSUM") as ps_s,
            tc.tile_pool(name="ps_av", bufs=2, space="PSUM") as ps_av,
            tc.tile_pool(name="ps_misc", bufs=2, space="PSUM") as ps_misc,
            tc.tile_pool(name="dram", bufs=1, space="DRAM") as dram,
        ):
            # ---- persistent state ----
            wq_sb = persist.tile([128, KS, 128], F16)
            wk_sb = persist.tile([128, KS, 128], F16)
            wv_sb = persist.tile([128, KS, 128], F16)
            wo_sb = persist.tile([128, KS, D], F16)
            nc.sync.dma_start(wq_sb[:], wq_d.rearrange("(o p) h -> p o h", p=128))

            qt_sb = persist.tile([128, BT], F16)  # [2 heads x 64, global t]
            # K^T zero-padded per head: [:, 0, t] rows 0-63 = head0 (rest 0),
            # [:, 1, t] rows 64-127 = head1 (rest 0)
            ktp_sb = persist.tile([128, 2, BT], F16)
            # V layout: [128 keys-in-block, B*NKB blocks, 2*(1+64)]
            #   per head h: cols 0:64 = V_h, col 64 = ones (denominator)
            v_sb = persist.tile([128, B * NKB, 2, DH + 1], F16)
            nc.vector.memset(v_sb[:, :, :, DH : DH + 1], 1.0)

            # per-partition selector scales for the padded K^T casts
            s01 = persist.tile([128, 1], F32)
            s10 = persist.tile([128, 1], F32)
            nc.vector.memset(s01[0:64], 1.0)
            nc.vector.memset(s01[64:128], 0.0)
            nc.vector.memset(s10[0:64], 0.0)
            nc.vector.memset(s10[64:128], 1.0)

            # bias broadcast [128, D] fp32 via PE ones-trick
            ones_col = persist.tile([1, 128], F32)
            nc.vector.memset(ones_col[:], 1.0)
            bo_sb = persist.tile([1, D], F32)
            nc.sync.dma_start(bo_sb[:], bo_d[None, :])
            bias_sb = persist.tile([128, D], F32)
            for nch in range(2):
                bps = ps_misc.tile([128, 512], F32, tag="misc")
                nc.tensor.matmul(
                    bps[:], ones_col[:], bo_sb[:, nch * 512 : (nch + 1) * 512]
                )
                nc.vector.tensor_copy(bias_sb[:, nch * 512 : (nch + 1) * 512], bps[:])

            # padded E2 selector (rows 0-63 <- r2[0], 64-127 <- r2[1]; rows
            # 2-127 of the moving operand are zero)
            e2_sb = persist.tile([128, 128], F16)
            nc.sync.dma_start(e2_sb[:], e2_d[:])
            r2hp = persist.tile([128, QC], F16)
            nc.vector.memset(r2hp[:], 0.0)

            # boundary triangle mask (both heads): cmask[p, h, j] = (j >= p)
            cmask_sb = persist.tile([128, 2, 128], F16)
            nc.sync.dma_start(cmask_sb[:], cmask_d[:])

            # ---- projection emission (as filler items) ----
            def emit_xt_dma(tcn):
                t0 = tcn * TC
                xt = xtp.tile([128, KS, TC], F16, tag="xt", name="xt")
                nc.sync.dma_start(
                    xt[:],
                    x_d[:, t0 : t0 + TC].rearrange("(o p) t -> p o t", p=128),
                )
                return xt

            def emit_q_group(xt, tcn):
                t0 = tcn * TC
                pp = ps_misc.tile([128, TC], F32, tag="misc", name="qp")
                for ks in range(KS):
                    nc.tensor.matmul(
                        pp[:], wq_sb[:, ks, :], xt[:, ks, :],
                        start=(ks == 0), stop=(ks == KS - 1),
                    )
                nc.scalar.copy(qt_sb[:, t0 : t0 + TC], pp[:])

            def emit_k_group(xt, tcn):
                t0 = tcn * TC
                pp = ps_misc.tile([128, TC], F32, tag="misc", name="kp")
                for ks in range(KS):
                    nc.tensor.matmul(
                        pp[:], wk_sb[:, ks, :], xt[:, ks, :],
                        start=(ks == 0), stop=(ks == KS - 1),
                    )
                nc.scalar.mul(ktp_sb[:, 0, t0 : t0 + TC], pp[:], s01[:])
                nc.vector.tensor_scalar_mul(ktp_sb[:, 1, t0 : t0 + TC], pp[:], s10[:])

            def emit_v_sub(xt, tcn, sub):
                vp = ps_misc.tile([128, 2, DH], F32, tag="misc", name="vp")
                for ks in range(KS):
                    nc.tensor.matmul(
                        vp[:],
                        xt[:, ks, sub * 128 : (sub + 1) * 128],
                        wv_sb[:, ks, :],
                        start=(ks == 0), stop=(ks == KS - 1),
                    )
                kbg = tcn * (TC // 128) + sub
                dst = v_sb[:, kbg, :, 0:DH]  # cols {0..63} u {65..128}
                nc.vector.tensor_copy(dst, vp[:])

            def proj_chunk_items(tcn):
                state = {}

                def first():
                    state["xt"] = emit_xt_dma(tcn)
                    emit_q_group(state["xt"], tcn)

                items = [first]
                items.append(lambda: emit_k_group(state["xt"], tcn))
                for sub in range(TC // 128):
                    items.append(
                        lambda s=sub: emit_v_sub(state["xt"], tcn, s)
                    )
                return items

            # ---- out-projection (as filler items) ----
            cc_ins = [dram.tile([N_CORES, 128, RB4], F16, name=f"cc_in{b}", tag=f"cc_in{b}") for b in range(B - 1)]
            cc_outs = [dram.tile([N_CORES, 128, RB4], F16, name=f"cc_out{b}", tag=f"cc_out{b}") for b in range(B - 1)]
            cc_ins_h = [dram.tile([N_CORES, 128, RB4 // 2], F16, name=f"cc_inh{i}", tag=f"cc_inh{i}") for i in range(2)]
            cc_outs_h = [dram.tile([N_CORES, 128, RB4 // 2], F16, name=f"cc_outh{i}", tag=f"cc_outh{i}") for i in range(2)]
            ao_sbs = []

            def emit_oproj_group(item):
                ob, oao, mb, nch = item
                t_in_ao = (mb * 128) % oao.shape[2]
                op = ps_misc.tile([128, 512], F32, tag="misc", name="op")
                for r in range(KS):
                    nc.tensor.matmul(
                        op[:],
                        oao[:, r, t_in_ao : t_in_ao + 128],
                        wo_sb[:, r, nch * 512 : (nch + 1) * 512],
                        start=(r == 0), stop=(r == KS - 1),
                    )
                osb = outp.tile([128, 512], F32, tag="osb", name="osb")
                nc.vector.tensor_tensor(
                    osb[:], op[:], bias_sb[:, nch * 512 : (nch + 1) * 512],
                    mybir.AluOpType.add,
                )
                nc.sync.dma_start(
                    out_d[ob, mb * 128 : (mb + 1) * 128,
                          nch * 512 : (nch + 1) * 512],
                    osb[:],
                )

            # ---- filler queue: keeps the PE fed between attention blocks.
            # Items carry a (b, qc) gate: not poppable before that position
            # (so an out-proj group never stalls the in-order PE behind its
            # AllToAll). cur_pos is updated by the attention loop. ----
            filler = []
            cur_pos = [0, 0]

            def pop_filler(n=1):
                popped = 0
                i = 0
                while popped < n and i < len(filler):
                    gate, fn = filler[i]
                    if gate <= (cur_pos[0], cur_pos[1]):
                        filler.pop(i)
                        fn()
                        popped += 1
                    else:
                        i += 1

            # batch 0 projections (+ chunks 4-5) emitted up front; first
            # x chunks' DMAs interleaved with the remaining weight DMAs so
            # the PE can start as soon as wq + xt0 land
            xt0 = emit_xt_dma(0)
            nc.sync.dma_start(wk_sb[:], wk_d.rearrange("(o p) h -> p o h", p=128))
            xt1 = emit_xt_dma(1)
            nc.sync.dma_start(wv_sb[:], wv_d.rearrange("(o p) h -> p o h", p=128))
            emit_q_group(xt0, 0)
            emit_k_group(xt0, 0)
            for sub in range(TC // 128):
                emit_v_sub(xt0, 0, sub)
            emit_q_group(xt1, 1)
            emit_k_group(xt1, 1)
            for sub in range(TC // 128):
                emit_v_sub(xt1, 1, sub)
            for tcn in range(2, 6):
                for it in proj_chunk_items(tcn):
                    it()
            nc.sync.dma_start(wo_sb[:], wo_d.rearrange("(r p) n -> p r n", p=128))
            for tcn in range(6, NTC):
                for it in proj_chunk_items(tcn):
                    filler.append(((0, 0), it))

            def emit_half_a2a(half):
                nc.gpsimd.collective_compute(
                    "AllToAll",
                    mybir.AluOpType.bypass,
                    replica_groups=[list(range(N_CORES))],
                    ins=[cc_ins_h[half][:]],
                    outs=[cc_outs_h[half][:]],
                )
                RBH = RB4 // 2
                ao_sb = persist.tile([128, KS, RBH], F16, name=f"aoh{half}", tag=f"aoh{half}")
                ao_sbs.append(ao_sb)
                nc.sync.dma_start(ao_sb[:], cc_outs_h[half].rearrange("r p t -> p r t"))
                gate = (B, 0)
                for nch in range(2):
                    filler.append(
                        (gate,
                         lambda a=ao_sb, h=half, n=nch: emit_oproj_group((B - 1, a, h, n)))
                    )

            # ---- attention ----
            pending_tail = []

            def emit_qc_tail_head(av0, av1):
                # DVE-only reciprocal chain, emitted right at qc end (no PE
                # involvement); reciprocal_approx_fast needs partition-0 ops
                d2a = tailp.tile([1, QC], F32, tag="d2a")
                d2b = tailp.tile([1, QC], F32, tag="d2b")
                nc.vector.tensor_copy(d2a[:], av0[DH : DH + 1, :])
                nc.vector.tensor_copy(d2b[:], av1[DH : DH + 1, :])
                r2a = tailp.tile([1, QC], F32, tag="r2a")
                r2b = tailp.tile([1, QC], F32, tag="r2b")
                nc.vector.reciprocal_approx_fast(r2a[:], d2a[:])
                nc.vector.reciprocal_approx_fast(r2b[:], d2b[:])
                nc.vector.tensor_copy(r2hp[0:1, :], r2a[:])
                nc.vector.tensor_copy(r2hp[64:65, :], r2b[:])

            def emit_qc_tail(b, qc, av0, av1):
                rb = ps_misc.tile([128, QC], F32, tag="misc", name="rb")
                nc.tensor.matmul(rb[:], e2_sb[:], r2hp[:])
                rb_sb = tailp.tile([128, QC], F16, tag="rbs")
                nc.vector.tensor_copy(rb_sb[:], rb[:])
                ctx2 = ctxp.tile([128, QC], F16, tag="ctx")
                nc.vector.tensor_tensor(
                    ctx2[0:64, :], av0[0:DH, :], rb_sb[0:64, :],
                    mybir.AluOpType.mult,
                )
                nc.vector.tensor_tensor(
                    ctx2[64:128, :], av1[0:DH, :], rb_sb[64:128, :],
                    mybir.AluOpType.mult,
                )
                if b < B - 1:
                    s0 = qc * QC // RB4
                    nc.sync.dma_start(
                        cc_ins[b][s0 : s0 + QC // RB4].rearrange("s p f -> p s f"),
                        ctx2[:].rearrange("p (s f) -> p s f", s=QC // RB4),
                    )
                else:
                    half, RBH = qc // 2, RB4 // 2
                    s0 = (qc % 2) * QC // RBH
                    nc.sync.dma_start(
                        cc_ins_h[half][s0 : s0 + QC // RBH].rearrange("s p f -> p s f"),
                        ctx2[:].rearrange("p (s f) -> p s f", s=QC // RBH),
                    )

            def emit_batch_a2a(b):
                nc.gpsimd.collective_compute(
                    "AllToAll",
                    mybir.AluOpType.bypass,
                    replica_groups=[list(range(N_CORES))],
                    ins=[cc_ins[b][:]],
                    outs=[cc_outs[b][:]],
                )
                ao_sb = persist.tile([128, KS, RB4], F16, name=f"ao{b}", tag=f"ao{b}")
                ao_sbs.append(ao_sb)
                nc.sync.dma_start(ao_sb[:], cc_outs[b].rearrange("r p t -> p r t"))
                for mb in range(RB4 // 128):
                    for nch in range(2):
                        filler.append(
                            ((b + 1, 1),
                             lambda a=ao_sb, m=mb, n=nch, bb=b: emit_oproj_group(
                                 (bb, a, m, n)
                             ))
                        )

            for b in range(B):
                for qc in range(NQC):
                    cur_pos[0], cur_pos[1] = b, qc
                    q0 = b * T + qc * QC
                    nkb = 4 * qc + 4

                    def emit_se(kb):
                        # scores + merged exp (+ triangle mask) for block kb
                        k0 = b * T + kb * KB
                        diag_i = kb - 4 * qc  # >= 0 for diagonal blocks
                        lo = max(0, diag_i) * 128  # live query range start
                        sp = ps_s.tile([128, 2, QC], F32, tag="s", name="sp")
                        for h in (0, 1):
                            nc.tensor.matmul(
                                sp[:, h, lo:QC],
                                ktp_sb[:, h, k0 : k0 + KB],
                                qt_sb[:, q0 + lo : q0 + QC],
                            )
                        e = ep.tile([128, 2, QC], F16, tag="e", name="e")
                        nc.scalar.activation(
                            e[:, :, lo:QC], sp[:, :, lo:QC],
                            mybir.ActivationFunctionType.Exp,
                            scale=0.125,
                        )
                        if diag_i >= 0:
                            nc.gpsimd.tensor_tensor(
                                e[:, :, lo : lo + 128],
                                e[:, :, lo : lo + 128],
                                cmask_sb[:],
                                mybir.AluOpType.mult,
                            )
                        return e, lo

                    # warm the S/exp pipeline, then flush the previous
                    # q-chunk's deferred tail (reads its av psum) BEFORE
                    # allocating this q-chunk's av tiles, then launch any
                    # collective whose inputs that tail produced
                    e_q = [emit_se(0)]
                    next_emit = 1
                    if nkb > 1:
                        e_q.append(emit_se(1))
                        next_emit = 2
                    while pending_tail:
                        pending_tail.pop(0)()
                    if qc == 0 and b > 0:
                        emit_batch_a2a(b - 1)
                    if b == B - 1 and qc == 2:
                        emit_half_a2a(0)

                    av0_full = ps_av.tile([128, QC], F32, tag="av", name="av0")
                    av1_full = ps_av.tile([128, QC], F32, tag="av", name="av1")
                    av0 = av0_full[: DH + 1]
                    av1 = av1_full[: DH + 1]

                    for kb in range(nkb):
                        kbg = b * NKB + kb
                        first, last = kb == 0, kb == nkb - 1
                        e_cur, lo = e_q.pop(0)
                        if kb > 0 and next_emit < nkb:
                            e_q.append(emit_se(next_emit))
                            next_emit += 1
                        nc.tensor.matmul(
                            av0[:, lo:QC], v_sb[:, kbg, 0, :], e_cur[:, 0, lo:QC],
                            start=first, stop=last,
                        )
                        if kb == 0 and next_emit < nkb:
                            # delay head1's first accumulation so the deferred
                            # division chain on DVE can release its av bank
                            e_q.append(emit_se(next_emit))
                            next_emit += 1
                        nc.tensor.matmul(
                            av1[:, lo:QC], v_sb[:, kbg, 1, :], e_cur[:, 1, lo:QC],
                            start=first, stop=last,
                        )
                        if kb % 2 == 1:
                            pop_filler(1)
                    emit_qc_tail_head(av0, av1)
                    pending_tail.append(
                        lambda b_=b, qc_=qc, a0=av0, a1=av1: emit_qc_tail(
                            b_, qc_, a0, a1
                        )
                    )

            while pending_tail:
                pending_tail.pop(0)()
            emit_half_a2a(1)
            cur_pos[0], cur_pos[1] = B, 0
            while filler:
                pop_filler(1)

    nc.compile()
    return nc


def _get_nc():
    if "nc" not in _CACHE:
        _CACHE["nc"] = _build()
    return _CACHE["nc"]


def prepare_in_maps(x, Wq, Wk, Wv, Wo, bo):
    x16 = np.ascontiguousarray(np.asarray(x, dtype=np.float32).reshape(BT, D).T).astype(np.float16)
    wo16 = np.asarray(Wo, dtype=np.float32).astype(np.float16)
    bo32 = np.ascontiguousarray(np.asarray(bo, dtype=np.float32))
    e2 = np.zeros((128, 128), dtype=np.float16)
    e2[0, 0:64] = 1.0
    e2[64, 64:128] = 1.0
    p = np.arange(128)[:, None]
    j = np.arange(128)[None, :]
    cmask = np.broadcast_to((j >= p).astype(np.float16)[:, None, :], (128, 2, 128))
    cmask = np.ascontiguousarray(cmask)
    in_maps = []
    for c in range(N_CORES):
        cs = slice(128 * c, 128 * (c + 1))
        in_maps.append(
            {
                "x": x16,
                "wq": np.ascontiguousarray(np.asarray(Wq, np.float32)[:, cs]).astype(np.float16),
                "wk": np.ascontiguousarray(np.asarray(Wk, np.float32)[:, cs]).astype(np.float16),
                "wv": np.ascontiguousarray(np.asarray(Wv, np.float32)[:, cs]).astype(np.float16),
                "wo": wo16,
                "bo": bo32,
                "e2": e2,
                "cmask": cmask,
            }
        )
    return in_maps


def kernel(x, Wq, Wk, Wv, Wo, bo, _trace=False):
    nc = _get_nc()
    in_maps = prepare_in_maps(x, Wq, Wk, Wv, Wo, bo)
    res = bass_utils.run_bass_kernel_spmd(
        nc, in_maps, list(range(N_CORES)), trace=_trace
    )
    if _trace:
        _CACHE["last_results"] = res
    out = np.empty((B, T, D), dtype=np.float32)
    rb4 = ROWS // B
    rbh = rb4 // 2
    for c in range(N_CORES):
        oc = res.results[c]["out"]  # [B, 256, D]
        for b in range(B - 1):
            out[b, rb4 * c : rb4 * (c + 1), :] = oc[b]
        # last batch was exchanged as two half-batch A2As with 128-row shards
        out[B - 1, rbh * c : rbh * (c + 1), :] = oc[B - 1, 0:rbh]
        out[B - 1, T // 2 + rbh * c : T // 2 + rbh * (c + 1), :] = oc[B - 1, rbh:]
    return out
